# revision 12
# baseline (speedup 1.0000x reference)
"""CapsNet-BCL Trainium2 kernel: 8-core SPMD Bass/Tile implementation.

Host algebra: fc1/fc2 have no nonlinearity between them, so
Weff[t] = fc2_w[t] @ fc1_w[t], beff[t] = fc2_w[t]@fc1_b[t]+fc2_b[t] and
h2 = x @ Weff[t].T + beff[t].  Only tasks r <= eval_t route (softmax mask
-10000 underflows to exactly 0 in fp32), so only route_weights[:, :eval_t+1]
is read.

Sharding: core k computes h2/sem for batches [8k, 8k+8); one AllGather of the
squashed sem; core c computes priors+routing for capsule c over all 64
batches.  The torch flat view vote(CAP,B,1,L)->(B,L,CAP) maps output batch b
to vote capsule b//8, so core c's vote is exactly what output batches
[8c,8c+8) need: each core emits its own output slice, no second collective.
"""

import sys

import numpy as np

if "/opt/trn_rl_repo" not in sys.path:
    sys.path.insert(0, "/opt/trn_rl_repo")

NTASKS = 10
CAP = 8
L = 256
D = 768
B = 64
N_CORES = 8
BL = B // N_CORES          # batches per core
TOK = BL * L               # tokens per core (2048)
KT = D // 128              # k tiles over D (6)
IT = (L * CAP) // 128      # i tiles over L*CAP (16)
NT = TOK // 512            # stage-1 moving chunks (4)

_CACHE = {}


def _build(A, use_cc=True):
    """Build the 8-core SPMD Bass program for A = eval_t+1 active tasks."""
    import concourse.bass as bass
    import concourse.tile as tile
    import concourse.mybir as mybir
    from concourse import bacc

    f32 = mybir.dt.float32
    f32r = mybir.dt.float32r
    Alu = mybir.AluOpType
    Act = mybir.ActivationFunctionType
    X = mybir.AxisListType.X

    nc = bacc.Bacc("TRN2", target_bir_lowering=False, debug=False,
                   num_devices=N_CORES)

    TC = NTASKS * CAP  # 80

    xT = nc.dram_tensor("xT", [D, TOK], f32r, kind="ExternalInput").ap()
    weffT = nc.dram_tensor("weffT", [D, TC], f32r, kind="ExternalInput").ap()
    beff_col = nc.dram_tensor("beff_col", [TC, 1], f32,
                              kind="ExternalInput").ap()
    rw = nc.dram_tensor("rw", [A, L * CAP, L], f32r, kind="ExternalInput").ap()
    wlT = nc.dram_tensor("wlT", [CAP, D], f32r, kind="ExternalInput").ap()
    lb_rep = nc.dram_tensor("lb_rep", [128, D], f32, kind="ExternalInput").ap()
    ident = nc.dram_tensor("ident", [128, 128], f32, kind="ExternalInput").ap()
    out = nc.dram_tensor("out", [BL, L, D], f32, kind="ExternalOutput").ap()

    NPAIR = (A + 1) // 2
    # collective chunks: {r0,r1} first (starts transposes early), rest after
    CH = [2, A - 2] if A > 2 else [A]
    CH0 = [0, 2]  # start r of each chunk
    sem_p = [nc.dram_tensor(f"sem_p{i}", [BL, n, L * CAP], f32).ap()
             for i, n in enumerate(CH)]
    gath_p = [nc.dram_tensor(f"gath_p{i}", [B, n, L * CAP], f32,
                             addr_space="Shared").ap()
              for i, n in enumerate(CH)]

    def gath_r(r):
        return (gath_p[0], r) if r < 2 else (gath_p[1], r - 2)
    voteT_dram = nc.dram_tensor("voteT_dram", [L, B], f32).ap()

    with tile.TileContext(nc) as tc:
        with tc.tile_pool(name="singles", bufs=1) as singles:
            # ---- constants ----
            weff_sb = singles.tile([128, KT * TC], f32r)
            nc.sync.dma_start(out=weff_sb,
                              in_=weffT.rearrange("(k p) c -> p k c", p=128))
            beff_sb = singles.tile([TC, 1], f32)
            nc.sync.dma_start(out=beff_sb, in_=beff_col)
            ident_sb = singles.tile([128, 128], f32)
            nc.sync.dma_start(out=ident_sb, in_=ident)
            wlT_sb = singles.tile([CAP, D], f32r)
            nc.sync.dma_start(out=wlT_sb, in_=wlT)
            lb_sb = singles.tile([128, D], f32)
            nc.sync.dma_start(out=lb_sb, in_=lb_rep)

            semT_sb = singles.tile([128, ((A + 1) // 2) * IT * 128], f32r)
            priors_sb = singles.tile([64, A * L], f32)

            # ===== Phase 1: semantic stage, batch-parallel ================
            # h2a psum [80 rows=(t,c), 512 tokens] (fp32r 1-pass, N=512),
            # PE-retranspose to [128 tok, 80], squash over t, write sem_local.
            with (
                tc.tile_pool(name="xT_pool", bufs=1) as xpool,
                tc.tile_pool(name="pA", bufs=2, space="PSUM") as pA,
                tc.tile_pool(name="pB", bufs=2, space="PSUM") as pB,
                tc.tile_pool(name="h2a_pool", bufs=2) as hapool,
                tc.tile_pool(name="sem_pool", bufs=3) as spool,
                tc.tile_pool(name="sq_pool", bufs=3) as qpool,
            ):
                xT_sb = xpool.tile([128, KT * TOK], f32r)
                for k in range(KT):
                    nc.sync.dma_start(
                        out=xT_sb[:, k * TOK:(k + 1) * TOK],
                        in_=xT[k * 128:(k + 1) * 128, :])

                for nt in range(NT):            # 4 chunks of 512 tokens
                    psa = pA.tile([TC, 512], f32, tag="psa")
                    for k in range(KT):
                        nc.tensor.matmul(
                            psa,
                            lhsT=weff_sb[:, k * TC:(k + 1) * TC],
                            rhs=xT_sb[:, k * TOK + nt * 512:
                                      k * TOK + (nt + 1) * 512],
                            start=(k == 0), stop=(k == KT - 1),
                        )
                    h2a = hapool.tile([TC, 512], f32, tag="h2a")
                    nc.vector.tensor_scalar_add(h2a, psa, beff_sb)
                    for s in range(4):          # 4 token sub-tiles of 128
                        m = nt * 4 + s
                        psb = pB.tile([128, TC], f32, tag="psb")
                        nc.tensor.transpose(
                            psb, in_=h2a[:, s * 128:(s + 1) * 128],
                            identity=ident_sb[:TC, :TC])
                        h2sq = spool.tile([128, TC], f32, tag="h2sq")
                        nc.scalar.activation(h2sq, psb, Act.Square)
                        sq = qpool.tile([128, CAP], f32, tag="sq")
                        nc.vector.tensor_reduce(
                            out=sq,
                            in_=h2sq.rearrange("p (t c) -> p c t", c=CAP),
                            axis=X, op=Alu.add)
                        rt = qpool.tile([128, CAP], f32, tag="rt")
                        nc.scalar.activation(rt, sq, Act.Sqrt)
                        den = qpool.tile([128, CAP], f32, tag="den")
                        nc.vector.tensor_scalar_add(den, sq, 1.0)
                        rden = qpool.tile([128, CAP], f32, tag="rden")
                        nc.vector.reciprocal(rden, den)
                        scal = qpool.tile([128, CAP], f32, tag="scal")
                        nc.vector.tensor_mul(scal, rt, rden)
                        sem = spool.tile([128, TC], f32, tag="sem")
                        scal_b = bass.AP(
                            tensor=scal.tensor, offset=scal.offset,
                            ap=[scal.ap[0], [1, CAP], [0, NTASKS]])
                        nc.vector.tensor_tensor(
                            out=sem.rearrange("p (t c) -> p c t", c=CAP),
                            in0=psb.rearrange("p (t c) -> p c t", c=CAP),
                            in1=scal_b, op=Alu.mult)
                        b_l, l0 = m // 2, (m % 2) * 128
                        for i, n in enumerate(CH):
                            dst = sem_p[i][b_l].rearrange(
                                "a (l c) -> l a c", c=CAP)[l0:l0 + 128]
                            nc.sync.dma_start(
                                out=dst,
                                in_=sem[:, CH0[i] * CAP:
                                        (CH0[i] + n) * CAP].rearrange(
                                    "p (t c) -> p t c", c=CAP))

            # ===== Phase 2: allgather sem =================================
            if use_cc:
                for i in range(len(CH)):
                    nc.gpsimd.collective_compute(
                        "AllGather", Alu.bypass,
                        replica_groups=[list(range(N_CORES))],
                        ins=[sem_p[i][:]], outs=[gath_p[i][:]])
            else:
                for i in range(len(CH)):
                    nc.sync.dma_start(out=gath_p[i][0:BL], in_=sem_p[i][:])

            # ===== Phase 3+4: transpose + priors (capsule-parallel) =======
            with (
                tc.tile_pool(name="gpool", bufs=2) as gpool,
                tc.tile_pool(name="pT", bufs=4, space="PSUM") as pT,
                tc.tile_pool(name="rw_pool", bufs=64) as rwpool,
                tc.tile_pool(name="pP", bufs=2, space="PSUM") as pP,
            ):
                def semT_slice(r, k):
                    pair, half = r // 2, r % 2
                    base = (pair * IT + k) * 128 + half * 64
                    return semT_sb[:, base:base + 64]

                for pair in range(NPAIR):
                    ra, rb = 2 * pair, 2 * pair + 1
                    g_sb = gpool.tile([128, L * CAP], f32, tag="g")
                    ta, ia = gath_r(ra)
                    nc.sync.dma_start(out=g_sb[0:64], in_=ta[:, ia, :])
                    if rb < A:
                        tb, ib = gath_r(rb)
                        nc.sync.dma_start(out=g_sb[64:128], in_=tb[:, ib, :])
                    for k in range(IT):
                        psT = pT.tile([128, 128], f32, tag="psT")
                        nc.tensor.transpose(
                            psT, in_=g_sb[:, k * 128:(k + 1) * 128],
                            identity=ident_sb)
                        nc.vector.tensor_copy(
                            out=semT_sb[:, (pair * IT + k) * 128:
                                        (pair * IT + k + 1) * 128],
                            in_=psT)
                    for r in (ra, rb):
                        if r >= A:
                            continue
                        pp = pP.tile([64, L], f32, tag="pp")
                        for k in range(IT):
                            rwt = rwpool.tile([128, L], f32r, tag="rw")
                            nc.sync.dma_start(
                                out=rwt, in_=rw[r, k * 128:(k + 1) * 128, :])
                            nc.tensor.matmul(
                                pp, lhsT=semT_slice(r, k), rhs=rwt,
                                start=(k == 0), stop=(k == IT - 1))
                        nc.vector.tensor_copy(
                            out=priors_sb[:, r * L:(r + 1) * L], in_=pp)

            # ===== Phase 5: routing (vectorized over r) ===================
            with (
                tc.tile_pool(name="route", bufs=1) as rp,
                tc.tile_pool(name="pV", bufs=2, space="PSUM") as pV,
            ):
                vote = rp.tile([64, L], f32)
                scr = rp.tile([64, L], f32)
                big = rp.tile([64, A * L], f32)
                outsq = rp.tile([64, L], f32)
                l1 = rp.tile([64, A], f32)
                l2 = rp.tile([64, A], f32)
                dots = rp.tile([64, A], f32)
                ex = rp.tile([64, A], f32)
                probs = rp.tile([64, A], f32)
                n2 = rp.tile([64, 1], f32)
                rt2 = rp.tile([64, 1], f32)
                den2 = rp.tile([64, 1], f32)
                rden2 = rp.tile([64, 1], f32)
                sc2 = rp.tile([64, 1], f32)
                mx = rp.tile([64, 1], f32)
                nmx = rp.tile([64, 1], f32)
                ssum = rp.tile([64, 1], f32)
                rsum = rp.tile([64, 1], f32)

                def squash_vote():
                    nc.vector.tensor_mul(scr, vote, vote)
                    nc.vector.tensor_reduce(out=n2, in_=scr, axis=X,
                                            op=Alu.add)
                    nc.scalar.activation(rt2, n2, Act.Sqrt)
                    nc.vector.tensor_scalar_add(den2, n2, 1.0)
                    nc.vector.reciprocal(rden2, den2)
                    nc.vector.tensor_mul(sc2, rt2, rden2)
                    nc.vector.tensor_scalar_mul(outsq, vote, sc2)

                def logit_update(l_prev, l_new):
                    tgt = l_new if l_prev is None else dots
                    for r in range(A):
                        nc.vector.scalar_tensor_tensor(
                            out=big[:, r * L:(r + 1) * L],
                            in0=priors_sb[:, r * L:(r + 1) * L],
                            scalar=1.0, in1=outsq,
                            op0=Alu.mult, op1=Alu.mult,
                            accum_out=tgt[:, r:r + 1])
                    if l_prev is not None:
                        nc.vector.tensor_add(l_new, dots, l_prev)

                def softmax_vote(l_in):
                    nc.vector.tensor_reduce(out=mx, in_=l_in, axis=X,
                                            op=Alu.max)
                    nc.vector.tensor_scalar_mul(nmx, mx, -1.0)
                    nc.scalar.activation(ex, l_in, Act.Exp, bias=nmx,
                                         accum_out=ssum)
                    nc.vector.reciprocal(rsum, ssum)
                    nc.vector.tensor_scalar_mul(probs, ex, rsum)
                    pr_b = bass.AP(
                        tensor=probs.tensor, offset=probs.offset,
                        ap=[probs.ap[0], [1, A], [0, L]])
                    nc.vector.tensor_tensor(
                        out=big.rearrange("p (r o) -> p r o", r=A),
                        in0=priors_sb.rearrange("p (r o) -> p r o", r=A),
                        in1=pr_b, op=Alu.mult)
                    nc.vector.tensor_reduce(
                        out=vote,
                        in_=big.rearrange("p (r o) -> p o r", r=A),
                        axis=X, op=Alu.add)

                # iter 1: uniform probs = 1/A
                nc.vector.tensor_reduce(
                    out=scr,
                    in_=priors_sb.rearrange("p (r o) -> p o r", r=A),
                    axis=X, op=Alu.add)
                nc.vector.tensor_scalar_mul(vote, scr, 1.0 / A)
                squash_vote()
                logit_update(None, l1)
                softmax_vote(l1)
                squash_vote()
                logit_update(l1, l2)
                softmax_vote(l2)

                # transpose vote [64, 256] -> voteT_dram [256, 64]
                vT_sb = rp.tile([128, 128], f32)
                for half in range(2):
                    pv = pV.tile([128, 64], f32, tag="pv")
                    nc.tensor.transpose(
                        pv, in_=vote[:, half * 128:(half + 1) * 128],
                        identity=ident_sb[:64, :64])
                    nc.vector.tensor_copy(
                        out=vT_sb[:, half * 64:(half + 1) * 64], in_=pv)
                    nc.sync.dma_start(
                        out=voteT_dram[half * 128:(half + 1) * 128],
                        in_=vT_sb[:, half * 64:(half + 1) * 64])
                # warm the PE clock (HAM) while voteT round-trips through
                # DRAM so the final matmuls run at 2.4 GHz
                for w in range(8):
                    pdum = pV.tile([64, 512], f32, tag="pdum")
                    nc.tensor.matmul(
                        pdum, lhsT=vote[:, 0:64],
                        rhs=priors_sb[:, 0:512],
                        start=True, stop=True)

            # ===== Phase 6: final linear ==================================
            # voteT_dram[o, b]; h_blT[cap, l] = voteT[(l%32)*8+cap,
            # b_l*8 + l//32].  vt2[cap, (lr, b)] loads with 256B bursts.
            with (
                tc.tile_pool(name="vt", bufs=1) as vtp,
                tc.tile_pool(name="pF", bufs=4, space="PSUM") as pF,
                tc.tile_pool(name="outp", bufs=3) as op_,
            ):
                vt2 = vtp.tile([CAP, 32 * B], f32)
                src = bass.AP(
                    tensor=voteT_dram.tensor, offset=voteT_dram.offset,
                    ap=[[B, CAP], [CAP * B, 32], [1, B]])
                nc.sync.dma_start(out=vt2, in_=src)
                # permute free layout (lr, b) -> (b, lr) during the f32r
                # convert, so each lhsT is a contiguous 128-col slice
                vt2r = vtp.tile([CAP, 32 * B], f32r)
                nc.vector.tensor_copy(
                    out=vt2r.rearrange("p (b lr) -> p b lr", lr=32),
                    in_=vt2.rearrange("p (lr b) -> p b lr", lr=32))
                NH = 2
                for b_l in range(BL):
                    for lt in range(2):
                        o_sb = op_.tile([128, D], f32, tag="o")
                        lhsT = vt2r[:, (b_l * CAP + lt * 4) * 32:
                                    (b_l * CAP + lt * 4) * 32 + 128]
                        for nh in range(NH):
                            pf = pF.tile([128, D // NH], f32, tag="pf")
                            nc.tensor.matmul(
                                pf, lhsT=lhsT,
                                rhs=wlT_sb[:, nh * (D // NH):
                                           (nh + 1) * (D // NH)],
                                start=True, stop=True)
                            nc.vector.tensor_add(
                                o_sb[:, nh * (D // NH):(nh + 1) * (D // NH)],
                                pf,
                                lb_sb[:, nh * (D // NH):(nh + 1) * (D // NH)])
                        nc.gpsimd.dma_start(
                            out=out[b_l, lt * 128:(lt + 1) * 128, :],
                            in_=o_sb)

    nc.compile()
    return nc


def _host_prep(x, fc1_w, fc1_b, fc2_w, fc2_b, route_weights, larger_w,
               larger_b, eval_t):
    A = int(eval_t) + 1
    f64 = np.float64
    weff = np.einsum("tcd,tdi->tci", fc2_w.astype(f64), fc1_w.astype(f64))
    beff = (np.einsum("tcd,td->tc", fc2_w.astype(f64), fc1_b.astype(f64))
            + fc2_b.astype(f64))
    weffT = np.ascontiguousarray(
        weff.reshape(NTASKS * CAP, D).T).astype(np.float32)
    beff_col = beff.reshape(NTASKS * CAP, 1).astype(np.float32)
    wlT = np.ascontiguousarray(larger_w[int(eval_t)].T).astype(np.float32)
    lb_rep = np.tile(larger_b[int(eval_t)].reshape(1, D), (128, 1)).astype(
        np.float32)
    ident = np.eye(128, dtype=np.float32)

    in_maps = []
    for c in range(N_CORES):
        xT_c = np.ascontiguousarray(
            x[c * BL:(c + 1) * BL].reshape(TOK, D).T).astype(np.float32)
        rw_c = np.ascontiguousarray(route_weights[c, :A]).astype(np.float32)
        in_maps.append({
            "xT": xT_c, "weffT": weffT, "beff_col": beff_col, "rw": rw_c,
            "wlT": wlT, "lb_rep": lb_rep, "ident": ident,
        })
    return A, in_maps


def kernel(**inputs):
    from concourse.bass_utils import run_bass_kernel_spmd

    A, in_maps = _host_prep(**inputs)
    if A not in _CACHE:
        _CACHE[A] = _build(A)
    nc = _CACHE[A]
    res = run_bass_kernel_spmd(nc, in_maps, core_ids=list(range(N_CORES)))
    return np.concatenate(
        [res.results[c]["out"] for c in range(N_CORES)], axis=0)



# revision 13
# speedup vs baseline: 1.2273x; 1.2273x over previous
"""CapsNet-BCL Trainium2 kernel: 8-core SPMD Bass/Tile implementation.

Host algebra: fc1/fc2 have no nonlinearity between them, so
Weff[t] = fc2_w[t] @ fc1_w[t], beff[t] = fc2_w[t]@fc1_b[t]+fc2_b[t] and
h2 = x @ Weff[t].T + beff[t].  Only tasks r <= eval_t route (softmax mask
-10000 underflows to exactly 0 in fp32), so only route_weights[:, :eval_t+1]
is read.

Sharding: core k computes h2/sem for batches [8k, 8k+8); one AllGather of the
squashed sem; core c computes priors+routing for capsule c over all 64
batches.  The torch flat view vote(CAP,B,1,L)->(B,L,CAP) maps output batch b
to vote capsule b//8, so core c's vote is exactly what output batches
[8c,8c+8) need: each core emits its own output slice, no second collective.

The contraction index i = l*CAP + c of sem/route_weights is permuted on host
to j = c*L + l, which makes the flat (t, j) sem offset equal (t*CAP+c)*L + l,
i.e. partition-major: phase 1 keeps h2 in its PE-native [80=(t,c), tok]
layout, does the squash-over-tasks via two tiny mask matmuls (sum of squares
over t, then broadcast of the scale back over t) and stores sem with one
clean uniform-stride DMA per 512-token chunk -- no per-tile transposes.

Routing numerics are chaotic (softmax logits reach ~200 => near-argmax), so
everything through the routing iterations stays fp32/f32r.  Only the final
linear (vote @ larger_w + larger_b) runs in fp16 (adds ~4e-4 rel err), with
the bias folded into the matmul via a ones row.
"""

import sys

import numpy as np

if "/opt/trn_rl_repo" not in sys.path:
    sys.path.insert(0, "/opt/trn_rl_repo")

NTASKS = 10
CAP = 8
L = 256
D = 768
B = 64
N_CORES = 8
BL = B // N_CORES          # batches per core
TOK = BL * L               # tokens per core (2048)
KT = D // 128              # k tiles over D (6)
IT = (L * CAP) // 128      # j tiles over L*CAP (16)
NT = TOK // 512            # phase-1 chunks of 512 tokens (4)

_CACHE = {}


def _build(A, use_cc=True):
    """Build the 8-core SPMD Bass program for A = eval_t+1 active tasks."""
    import concourse.bass as bass
    import concourse.tile as tile
    import concourse.mybir as mybir
    from concourse import bacc

    f32 = mybir.dt.float32
    f32r = mybir.dt.float32r
    f16 = mybir.dt.float16
    Alu = mybir.AluOpType
    Act = mybir.ActivationFunctionType
    X = mybir.AxisListType.X

    nc = bacc.Bacc("TRN2", target_bir_lowering=False, debug=False,
                   num_devices=N_CORES)

    TC = NTASKS * CAP  # 80
    AC = A * CAP       # active (t,c) rows

    xT = nc.dram_tensor("xT", [D, TOK], f32r, kind="ExternalInput").ap()
    weffT = nc.dram_tensor("weffT", [D, TC], f32r, kind="ExternalInput").ap()
    beff_col = nc.dram_tensor("beff_col", [TC, 1], f32,
                              kind="ExternalInput").ap()
    rw = nc.dram_tensor("rw", [A, L * CAP, L], f32r, kind="ExternalInput").ap()
    maskA = nc.dram_tensor("maskA", [TC, CAP], f32r,
                           kind="ExternalInput").ap()
    maskAT = nc.dram_tensor("maskAT", [CAP, TC], f32r,
                            kind="ExternalInput").ap()
    wlb = nc.dram_tensor("wlb", [CAP + 1, D], f16, kind="ExternalInput").ap()
    identr = nc.dram_tensor("identr", [128, 128], f32,
                            kind="ExternalInput").ap()
    identh = nc.dram_tensor("identh", [128, 128], f16,
                            kind="ExternalInput").ap()
    out = nc.dram_tensor("out", [BL, L, D], f32, kind="ExternalOutput").ap()

    sem_p = nc.dram_tensor("sem_p", [BL, A * L * CAP], f32).ap()
    gath = nc.dram_tensor("gath", [B, A * L * CAP], f32,
                          addr_space="Shared").ap()
    warm_in = nc.dram_tensor("warm_in", [1, 128], f32).ap()
    warm_out = nc.dram_tensor("warm_out", [N_CORES, 128], f32,
                              addr_space="Shared").ap()
    voteT_dram = nc.dram_tensor("voteT_dram", [L, B], f16).ap()

    NPAIR = (A + 1) // 2

    with tile.TileContext(nc) as tc:
        with tc.tile_pool(name="singles", bufs=1) as singles:
            # ---- constants (sync queue: weff first, then per-chunk xT,
            # then rw; scalar queue: the small ones) ----
            weff_sb = singles.tile([128, KT * TC], f32r)
            nc.sync.dma_start(out=weff_sb,
                              in_=weffT.rearrange("(k p) c -> p k c", p=128))
            beff_sb = singles.tile([TC, 1], f32)
            nc.scalar.dma_start(out=beff_sb, in_=beff_col)
            maskA_sb = singles.tile([TC, CAP], f32r)
            nc.scalar.dma_start(out=maskA_sb, in_=maskA)
            maskAT_sb = singles.tile([CAP, TC], f32r)
            nc.scalar.dma_start(out=maskAT_sb, in_=maskAT)
            identf_sb = singles.tile([128, 128], f32)
            nc.scalar.dma_start(out=identf_sb, in_=identr)
            identh_sb = singles.tile([128, 128], f16)
            nc.scalar.dma_start(out=identh_sb, in_=identh)
            wlb_sb = singles.tile([CAP + 1, D], f16)
            nc.scalar.dma_start(out=wlb_sb, in_=wlb)

            rw_sb = singles.tile([128, A * IT * L], f32r)
            vacc_a = singles.tile([64, L], f32)
            vacc_b = singles.tile([64, L], f32)
            semT_sb = singles.tile([128, NPAIR * IT * 128], f32r)
            priors_sb = singles.tile([64, A * L], f32)

            # warm up the collectives firmware + rendezvous the cores while
            # phase 1 runs, so the real AllGather triggers into a warm ncfw
            WARM_AG = False
            if use_cc and WARM_AG:
                nc.gpsimd.collective_compute(
                    "AllGather", Alu.bypass,
                    replica_groups=[list(range(N_CORES))],
                    ins=[warm_in[:]], outs=[warm_out[:]])

            # ===== Phase 1: semantic stage, batch-parallel ================
            # psum h2 [80=(t,c), 512 tok]; squash over t via mask matmuls:
            # msq[c,tok] = sum_t h2^2, srep[(t,c),tok] = scale[c,tok]; then
            # sem = h2 * srep, stored as sem_p[b_l, (t*8+c)*256 + l].
            with (
                tc.tile_pool(name="xc_pool", bufs=2) as xpool,
                tc.tile_pool(name="pA", bufs=2, space="PSUM") as pA,
                tc.tile_pool(name="pM", bufs=2, space="PSUM") as pM,
                tc.tile_pool(name="pS", bufs=2, space="PSUM") as pS,
                tc.tile_pool(name="h_pool", bufs=2) as hpool,
                tc.tile_pool(name="q_pool", bufs=2) as qpool,
                tc.tile_pool(name="sem_pool", bufs=2) as spool,
            ):
                xT_r = xT.rearrange("(k p) t -> p k t", p=128)
                xcs = []
                for nt in range(NT):
                    xc = xpool.tile([128, KT * 512], f32r, tag="xc")
                    nc.sync.dma_start(
                        out=xc.rearrange("p (k n) -> p k n", k=KT),
                        in_=xT_r[:, :, nt * 512:(nt + 1) * 512])
                    xcs.append(xc)

                # rw prefetch queued behind xT on the sync ring
                for r in range(A):
                    nc.sync.dma_start(
                        out=rw_sb[:, r * IT * L:(r + 1) * IT * L].rearrange(
                            "p (k o) -> p k o", k=IT),
                        in_=rw[r].rearrange("(k p) o -> p k o", p=128))

                for nt in range(NT):
                    xc = xcs[nt]
                    psa = pA.tile([TC, 512], f32, tag="psa")
                    for k in range(KT):
                        nc.tensor.matmul(
                            psa,
                            lhsT=weff_sb[:, k * TC:(k + 1) * TC],
                            rhs=xc[:, k * 512:(k + 1) * 512],
                            start=(k == 0), stop=(k == KT - 1),
                        )
                    h2a = hpool.tile([TC, 512], f32, tag="h2a")
                    nc.vector.tensor_scalar_add(h2a, psa, beff_sb)
                    h2sq = hpool.tile([TC, 512], f32r, tag="h2sq")
                    nc.scalar.activation(h2sq, h2a, Act.Square)
                    msq = pM.tile([CAP, 512], f32, tag="msq")
                    nc.tensor.matmul(msq, lhsT=maskA_sb, rhs=h2sq,
                                     start=True, stop=True)
                    den = qpool.tile([CAP, 512], f32, tag="den")
                    nc.vector.tensor_scalar_add(den, msq, 1.0)
                    rden = qpool.tile([CAP, 512], f32, tag="rden")
                    nc.vector.reciprocal(rden, den)
                    rt = qpool.tile([CAP, 512], f32, tag="rt")
                    nc.scalar.activation(rt, msq, Act.Sqrt)
                    scal = qpool.tile([CAP, 512], f32r, tag="scal")
                    nc.vector.tensor_mul(scal, rt, rden)
                    srep = pS.tile([AC, 512], f32, tag="srep")
                    nc.tensor.matmul(srep, lhsT=maskAT_sb[:, 0:AC], rhs=scal,
                                     start=True, stop=True)
                    sem80 = spool.tile([AC, 512], f32, tag="sem80")
                    nc.vector.tensor_tensor(out=sem80, in0=h2a[0:AC],
                                            in1=srep, op=Alu.mult)
                    # sem_p[b_l, p*256 + l] for p=(t*8+c): uniform stride
                    nc.scalar.dma_start(
                        out=sem_p.rearrange("b (p l) -> p b l",
                                            l=L)[:, 2 * nt:2 * nt + 2],
                        in_=sem80.rearrange("p (b l) -> p b l", b=2))

            # ===== Phase 2: allgather sem =================================
            if use_cc:
                nc.gpsimd.collective_compute(
                    "AllGather", Alu.bypass,
                    replica_groups=[list(range(N_CORES))],
                    ins=[sem_p[:]], outs=[gath[:]])
            else:
                nc.sync.dma_start(out=gath[0:BL], in_=sem_p[:])

            # ===== Phase 3+4: transpose + priors (capsule-parallel) =======
            va = None
            with (
                tc.tile_pool(name="gpool", bufs=2) as gpool,
                tc.tile_pool(name="pT", bufs=4, space="PSUM") as pT,
                tc.tile_pool(name="pP", bufs=2, space="PSUM") as pP,
            ):
                def semT_slice(r, k):
                    pair, half = r // 2, r % 2
                    base = (pair * IT + k) * 128 + half * 64
                    return semT_sb[:, base:base + 64]

                for pair in range(NPAIR):
                    ra, rb = 2 * pair, 2 * pair + 1
                    g_sb = gpool.tile([128, L * CAP], f32, tag="g")
                    nc.scalar.dma_start(
                        out=g_sb[0:64],
                        in_=gath[:, ra * L * CAP:(ra + 1) * L * CAP])
                    if rb < A:
                        nc.scalar.dma_start(
                            out=g_sb[64:128],
                            in_=gath[:, rb * L * CAP:(rb + 1) * L * CAP])
                    for k in range(IT):
                        psT = pT.tile([128, 128], f32, tag="psT")
                        nc.tensor.transpose(
                            psT, in_=g_sb[:, k * 128:(k + 1) * 128],
                            identity=identf_sb)
                        nc.vector.tensor_copy(
                            out=semT_sb[:, (pair * IT + k) * 128:
                                        (pair * IT + k + 1) * 128],
                            in_=psT)
                    for r in (ra, rb):
                        if r >= A:
                            continue
                        pp = pP.tile([64, L], f32, tag="pp")
                        for k in range(IT):
                            nc.tensor.matmul(
                                pp, lhsT=semT_slice(r, k),
                                rhs=rw_sb[:, (r * IT + k) * L:
                                          (r * IT + k + 1) * L],
                                start=(k == 0), stop=(k == IT - 1))
                        nc.scalar.activation(
                            priors_sb[:, r * L:(r + 1) * L], pp, Act.Copy)
                        # running sum over r feeds iteration-1's vote
                        vb = vacc_a if r % 2 == 0 else vacc_b
                        if r == 0:
                            nc.vector.tensor_copy(out=vb, in_=pp)
                        else:
                            nc.vector.tensor_add(vb, va, pp)
                        va = vb

            # ===== Phase 5: routing (fp32, 3 iterations) ==================
            with (
                tc.tile_pool(name="route", bufs=1) as rp,
                tc.tile_pool(name="scr_pool", bufs=2) as scp,
                tc.tile_pool(name="pW", bufs=2, space="PSUM") as pW,
            ):
                outsq = rp.tile([64, L], f32)
                outsq2 = rp.tile([64, L], f32)
                v2a = rp.tile([64, L], f32)
                v2b = rp.tile([64, L], f32)
                v3a = rp.tile([64, L], f32)
                v3b = rp.tile([64, L], f32)
                vote_f16 = rp.tile([64, L], f16)
                l1 = rp.tile([64, A], f32)
                l2 = rp.tile([64, A], f32)
                dtmp = rp.tile([64, A], f32)
                ex = rp.tile([64, A], f32)
                probs = rp.tile([64, A], f32)
                probs2 = rp.tile([64, A], f32)
                n2 = rp.tile([64, 1], f32)
                rt2 = rp.tile([64, 1], f32)
                den2 = rp.tile([64, 1], f32)
                rden2 = rp.tile([64, 1], f32)
                sc2 = rp.tile([64, 1], f32)
                mx = rp.tile([64, 1], f32)
                nmx = rp.tile([64, 1], f32)
                ssum = rp.tile([64, 1], f32)
                rsum = rp.tile([64, 1], f32)

                def warm_pe(t):
                    # tiny dep-gated matmul keeps HAM from re-throttling
                    m = min(t.shape[1], 64)
                    pd = pW.tile([64, 64], f32, tag="pd")
                    nc.tensor.matmul(pd[0:m, 0:m], lhsT=t[:, 0:m],
                                     rhs=t[:, 0:m], start=True, stop=True)

                def squash(vin, scale, osq):
                    scr = scp.tile([64, L], f32, tag="scr")
                    nc.scalar.activation(scr, vin, Act.Square, scale=scale,
                                         accum_out=n2)
                    nc.scalar.activation(rt2, n2, Act.Sqrt)
                    nc.vector.tensor_scalar_add(den2, n2, 1.0)
                    nc.vector.reciprocal(rden2, den2)
                    nc.vector.scalar_tensor_tensor(
                        out=sc2, in0=rt2, scalar=scale, in1=rden2,
                        op0=Alu.mult, op1=Alu.mult)
                    nc.scalar.activation(osq, vin, Act.Copy, scale=sc2)

                def dots(osq, l_prev, l_new):
                    tgt = l_new if l_prev is None else dtmp
                    for r in range(A):
                        scr = scp.tile([64, L], f32, tag="scr")
                        nc.vector.scalar_tensor_tensor(
                            out=scr,
                            in0=priors_sb[:, r * L:(r + 1) * L],
                            scalar=1.0, in1=osq,
                            op0=Alu.mult, op1=Alu.mult,
                            accum_out=tgt[:, r:r + 1])
                    if l_prev is not None:
                        nc.vector.tensor_add(l_new, dtmp, l_prev)

                def softmax(l_in, pr):
                    nc.vector.tensor_reduce(out=mx, in_=l_in, axis=X,
                                            op=Alu.max)
                    nc.vector.tensor_scalar_mul(nmx, mx, -1.0)
                    nc.scalar.activation(ex, l_in, Act.Exp, bias=nmx,
                                         accum_out=ssum)
                    nc.vector.reciprocal(rsum, ssum)
                    nc.vector.tensor_scalar_mul(pr, ex, rsum)

                def weighted_vote(pr, ta, tb, final):
                    cur = ta
                    nc.vector.tensor_scalar_mul(
                        cur, priors_sb[:, 0:L], pr[:, 0:1])
                    for r in range(1, A):
                        nxt = tb if cur is ta else ta
                        if r == A - 1 and final is not None:
                            nxt = final
                        nc.vector.scalar_tensor_tensor(
                            out=nxt, in0=priors_sb[:, r * L:(r + 1) * L],
                            scalar=pr[:, r:r + 1], in1=cur,
                            op0=Alu.mult, op1=Alu.add)
                        cur = nxt
                    return cur

                # iter 1: probs uniform 1/A; vote1 = va/A (folded into scale)
                squash(va, 1.0 / A, outsq)
                warm_pe(outsq)
                dots(outsq, None, l1)
                warm_pe(l1)
                softmax(l1, probs)
                warm_pe(probs)
                v2 = weighted_vote(probs, v2a, v2b, None)
                warm_pe(v2)
                squash(v2, 1.0, outsq2)
                warm_pe(outsq2)
                dots(outsq2, l1, l2)
                warm_pe(l2)
                softmax(l2, probs2)
                warm_pe(probs2)
                weighted_vote(probs2, v3a, v3b, vote_f16)

                # ===== Phase 6: final linear (fp16) =======================
                with (
                    tc.tile_pool(name="vt", bufs=1) as vtp,
                    tc.tile_pool(name="pV", bufs=2, space="PSUM") as pV,
                    tc.tile_pool(name="pF", bufs=4, space="PSUM") as pF,
                    tc.tile_pool(name="outp", bufs=3) as op_,
                ):
                    vT_sb = vtp.tile([128, 128], f16)
                    for half in range(2):
                        pv = pV.tile([128, 64], f16, tag="pv")
                        nc.tensor.transpose(
                            pv, in_=vote_f16[:, half * 128:(half + 1) * 128],
                            identity=identh_sb[:64, :64])
                        nc.vector.tensor_copy(
                            out=vT_sb[:, half * 64:(half + 1) * 64], in_=pv)
                        nc.sync.dma_start(
                            out=voteT_dram[half * 128:(half + 1) * 128],
                            in_=vT_sb[:, half * 64:(half + 1) * 64])
                    # vt2[cap, (lr, b)] <- voteT[lr*8+cap, b]
                    vt2 = vtp.tile([CAP, 32 * B], f16)
                    src = bass.AP(
                        tensor=voteT_dram.tensor, offset=voteT_dram.offset,
                        ap=[[B, CAP], [CAP * B, 32], [1, B]])
                    nc.sync.dma_start(out=vt2, in_=src)
                    # permute (lr, b) -> (b, lr); row 8 = ones for the bias
                    vt2f = vtp.tile([CAP + 1, 32 * B], f16)
                    nc.vector.memset(vt2f, 1.0)
                    nc.vector.tensor_copy(
                        out=vt2f[0:CAP].rearrange("p (b lr) -> p b lr",
                                                  lr=32),
                        in_=vt2.rearrange("p (lr b) -> p b lr", lr=32))
                    NH = 2
                    for b_l in range(BL):
                        for lt in range(2):
                            o_sb = op_.tile([128, D], f32, tag="o")
                            lhsT = vt2f[:, (b_l * CAP + lt * 4) * 32:
                                        (b_l * CAP + lt * 4) * 32 + 128]
                            for nh in range(NH):
                                pf = pF.tile([128, D // NH], f32, tag="pf")
                                nc.tensor.matmul(
                                    pf, lhsT=lhsT,
                                    rhs=wlb_sb[:, nh * (D // NH):
                                               (nh + 1) * (D // NH)],
                                    start=True, stop=True)
                                dst = o_sb[:, nh * (D // NH):
                                           (nh + 1) * (D // NH)]
                                if (b_l * 2 + lt + nh) % 2 == 0:
                                    nc.vector.tensor_copy(out=dst, in_=pf)
                                else:
                                    nc.scalar.activation(dst, pf, Act.Copy)
                            nc.gpsimd.dma_start(
                                out=out[b_l, lt * 128:(lt + 1) * 128, :],
                                in_=o_sb)

    nc.compile()
    return nc


def _host_prep(x, fc1_w, fc1_b, fc2_w, fc2_b, route_weights, larger_w,
               larger_b, eval_t):
    A = int(eval_t) + 1
    f64 = np.float64
    weff = np.einsum("tcd,tdi->tci", fc2_w.astype(f64), fc1_w.astype(f64))
    beff = (np.einsum("tcd,td->tc", fc2_w.astype(f64), fc1_b.astype(f64))
            + fc2_b.astype(f64))
    weffT = np.ascontiguousarray(
        weff.reshape(NTASKS * CAP, D).T).astype(np.float32)
    beff_col = beff.reshape(NTASKS * CAP, 1).astype(np.float32)
    wl = larger_w[int(eval_t)].astype(np.float32)          # [D, CAP]
    wlb = np.concatenate(
        [wl.T, larger_b[int(eval_t)].reshape(1, D).astype(np.float32)],
        axis=0).astype(np.float16)                         # [CAP+1, D]
    identr = np.eye(128, dtype=np.float32)
    identh = np.eye(128, dtype=np.float16)
    maskA = np.zeros((NTASKS * CAP, CAP), dtype=np.float32)
    for t in range(NTASKS):
        for c in range(CAP):
            maskA[t * CAP + c, c] = 1.0
    maskAT = np.ascontiguousarray(maskA.T)

    in_maps = []
    for c in range(N_CORES):
        xT_c = np.ascontiguousarray(
            x[c * BL:(c + 1) * BL].reshape(TOK, D).T).astype(np.float32)
        # permute contraction index i = l*CAP + cap -> j = cap*L + l
        rw_c = route_weights[c, :A].reshape(A, L, CAP, L)
        rw_c = np.ascontiguousarray(
            rw_c.transpose(0, 2, 1, 3).reshape(A, L * CAP, L)
        ).astype(np.float32)
        in_maps.append({
            "xT": xT_c, "weffT": weffT, "beff_col": beff_col, "rw": rw_c,
            "maskA": maskA, "maskAT": maskAT, "wlb": wlb,
            "identr": identr, "identh": identh,
        })
    return A, in_maps


def kernel(**inputs):
    from concourse.bass_utils import run_bass_kernel_spmd

    A, in_maps = _host_prep(**inputs)
    if A not in _CACHE:
        _CACHE[A] = _build(A)
    nc = _CACHE[A]
    res = run_bass_kernel_spmd(nc, in_maps, core_ids=list(range(N_CORES)))
    return np.concatenate(
        [res.results[c]["out"] for c in range(N_CORES)], axis=0)
